# revision 1
# baseline (speedup 1.0000x reference)
"""Trainium2 Bass kernel for nn_SubspaceLinopFactory (subspace NUDFT forward op).

Math (reference):
  s[a,c,h,w] = x[a,h,w] * mps[c,h,w]
  E[r,k,(h,w)] = exp(-i*(trj[r,0,k]*gy[h] + trj[r,1,k]*gx[w]))   (separable)
  y[a,r,c,k] = sum_hw E * s
  z[r,t,c,k] = sum_a phi[a,t] * y[a,r,c,k] * sqrt_dcf[r,k]
  out[t,c,k] = z[subsamp_idx[t], t, c, k]

Sharding: trajectory r -> core r (R == 8 == n_cores). Each core computes
z[t,c,k] for all t with its own r; host gathers rows where subsamp_idx[t]==r.

Device pipeline per core (separable NUDFT, fp16 matmul operands / f32 accum):
  - trig tables per k-chunk: host stages packed phase inputs in "turns"
    ([sin|cos] halves; the cos half pre-shifted by a quarter turn), ScalarE
    Copy applies the per-partition gy/2pi scale, VectorE int32-cast roundtrip
    gives frac = m-round(m) in [-.5,.5], ScalarE Sin(2pi*frac) -> fp16 tables.
  - stage 1 (TensorE, fp16): P[(a,c,h),k] = sum_w sT[w,ach]*(dcf*cos_x)[w,k],
    Q likewise with sin_x. 6 m-tiles x 512-wide k-chunks, PSUM f32.
  - ScalarE casts P,Q PSUM->SBUF fp16; VectorE products A=cy*P, B=sy*Q,
    C=cy*Q, D=sy*P (fp16 2x mode).
  - h-reduction (TensorE): +-1 selector matmuls contract (ac,h) partitions:
    y_re[ac,k] = sum_h A-B, y_im = -(C+D), PSUM-accumulated over m-tiles.
  - phi expansion (TensorE): z[(t,c),k] = phiT.T @ y  (rows = t*4+c = 128).
  - z_re, z_im [128,1024] f32 -> host gathers into [T,C,K] complex64.
"""
import numpy as np

A, T, C, R, D, K, H, W = 3, 32, 4, 8, 2, 1024, 64, 64
N_CORES = 8
ACH = A * C * H          # 768
MT = ACH // 128          # 6 m-tiles
KC = 512                 # k-chunk (one PSUM bank of f32)
NKC = K // KC            # 2

_CACHE = {}


def _build_nc():
    import concourse.bacc as bacc
    import concourse.tile as tile
    import concourse.mybir as mybir

    AF = mybir.ActivationFunctionType
    OP = mybir.AluOpType
    F32 = mybir.dt.float32
    F16 = mybir.dt.float16
    I32 = mybir.dt.int32
    TWO_PI = float(2 * np.pi)

    nc = bacc.Bacc(None, target_bir_lowering=False)

    # batched inputs: big64 = [txr2 | dcf2 | xr | mr] on 64 partitions,
    # big128 = [tyr2 | pp] on 128, sel = [selp | selm] fp16, phit fp16.
    W64 = 2 * K + 2 * K + ACH + ACH  # 5632
    d_b64 = nc.dram_tensor("b64", [64, W64], F32, kind="ExternalInput")
    d_b128 = nc.dram_tensor("b128", [128, 2 * K + 2], F32, kind="ExternalInput")
    d_sel = nc.dram_tensor("sel", [128, 24 * MT], F16, kind="ExternalInput")
    d_phit = nc.dram_tensor("phit", [12, 128], F16, kind="ExternalInput")
    d_zre = nc.dram_tensor("zre", [128, K], F32, kind="ExternalOutput")
    d_zim = nc.dram_tensor("zim", [128, K], F32, kind="ExternalOutput")

    with tile.TileContext(nc) as tc:
        with (
            tc.tile_pool(name="cst", bufs=1) as cst,
            tc.tile_pool(name="tabw", bufs=2) as tabw,
            tc.tile_pool(name="tbl", bufs=2) as tblp,
            tc.tile_pool(name="work", bufs=3) as work,
            tc.tile_pool(name="psA", bufs=2, space="PSUM") as psA,
            tc.tile_pool(name="psY", bufs=1, space="PSUM") as psY,
            tc.tile_pool(name="psZ", bufs=1, space="PSUM") as psZ,
        ):
            b64 = cst.tile([64, W64], F32)
            b128 = cst.tile([128, 2 * K + 2], F32)
            sel = cst.tile([128, 24 * MT], F16)
            phit = cst.tile([12, 128], F16)
            nc.sync.dma_start(b64[:], d_b64[:])
            nc.sync.dma_start(b128[:], d_b128[:])
            nc.sync.dma_start(sel[:], d_sel[:])
            nc.sync.dma_start(phit[:], d_phit[:])

            txr2 = b64[:, 0:2 * K].rearrange("p (s k) -> p s k", s=2)
            dcf2 = b64[:, 2 * K:4 * K].rearrange("p (s k) -> p s k", s=2)
            xr = b64[:, 4 * K:4 * K + ACH]
            mr = b64[:, 4 * K + ACH:4 * K + 2 * ACH]
            tyr2 = b128[:, 0:2 * K].rearrange("p (s k) -> p s k", s=2)
            ppy = b128[:, 2 * K:2 * K + 1]
            ppx = b128[:64, 2 * K + 1:2 * K + 2]

            # sT = x_rep * mps_rep  -> fp16 [64, ACH]
            sT = cst.tile([64, ACH], F16)
            nc.vector.tensor_tensor(sT[:], xr[:], mr[:], OP.mult)

            selp = sel[:, 0:12 * MT]
            selm = sel[:, 12 * MT:24 * MT]

            zout_re = cst.tile([128, K], F32)
            zout_im = cst.tile([128, K], F32)

            def trig_chunk(src, scale_ap, P, kc, name, out_dt):
                """[P, 2, KC] fp16 table chunk: [:,0,:]=sin, [:,1,:]=cos."""
                ks = slice(kc * KC, (kc + 1) * KC)
                m = tabw.tile([P, 2, KC], F32, tag=f"m{name}")
                nc.scalar.activation(m[:], src[:, :, ks], AF.Copy, scale=scale_ap)
                mi = tabw.tile([P, 2, KC], I32, tag=f"mi{name}")
                nc.vector.tensor_copy(mi[:], m[:])
                mf = tabw.tile([P, 2, KC], F32, tag=f"mf{name}")
                nc.vector.tensor_copy(mf[:], mi[:])
                fr = tabw.tile([P, 2, KC], F32, tag=f"fr{name}")
                nc.vector.tensor_tensor(fr[:], m[:], mf[:], OP.subtract)
                o = tblp.tile([P, 2, KC], out_dt, tag=f"tbl{name}")
                nc.scalar.activation(o[:], fr[:], AF.Sin, scale=TWO_PI)
                return o

            for kc in range(NKC):
                ks = slice(kc * KC, (kc + 1) * KC)
                xt = trig_chunk(txr2, ppx, 64, kc, "x", F32)
                xtd = tblp.tile([64, 2, KC], F16, tag="xtd")
                nc.vector.tensor_tensor(xtd[:], xt[:], dcf2[:, :, ks], OP.mult)
                yt = trig_chunk(tyr2, ppy, 128, kc, "y", F16)

                yre = psY.tile([12, KC], F32, tag="yre")
                yim = psY.tile([12, KC], F32, tag="yim")
                for j in range(MT):
                    js = slice(j * 128, (j + 1) * 128)
                    p_ps = psA.tile([128, KC], F32, tag="p")
                    q_ps = psA.tile([128, KC], F32, tag="q")
                    nc.tensor.matmul(p_ps[:], sT[:, js], xtd[:, 1, :],
                                     start=True, stop=True)
                    nc.tensor.matmul(q_ps[:], sT[:, js], xtd[:, 0, :],
                                     start=True, stop=True)
                    pc = work.tile([128, KC], F16, tag="pc")
                    qc = work.tile([128, KC], F16, tag="qc")
                    nc.scalar.copy(pc[:], p_ps[:])
                    nc.scalar.copy(qc[:], q_ps[:])
                    prodA = work.tile([128, KC], F16, tag="A")
                    prodB = work.tile([128, KC], F16, tag="B")
                    prodC = work.tile([128, KC], F16, tag="C")
                    prodD = work.tile([128, KC], F16, tag="D")
                    nc.vector.tensor_tensor(prodA[:], pc[:], yt[:, 1, :], OP.mult)
                    nc.vector.tensor_tensor(prodB[:], qc[:], yt[:, 0, :], OP.mult)
                    nc.vector.tensor_tensor(prodC[:], qc[:], yt[:, 1, :], OP.mult)
                    nc.vector.tensor_tensor(prodD[:], pc[:], yt[:, 0, :], OP.mult)
                    sj = slice(j * 12, (j + 1) * 12)
                    nc.tensor.matmul(yre[:], selp[:, sj], prodA[:],
                                     start=(j == 0), stop=False,
                                     skip_group_check=True)
                    nc.tensor.matmul(yre[:], selm[:, sj], prodB[:],
                                     start=False, stop=(j == MT - 1),
                                     skip_group_check=True)
                    nc.tensor.matmul(yim[:], selm[:, sj], prodC[:],
                                     start=(j == 0), stop=False,
                                     skip_group_check=True)
                    nc.tensor.matmul(yim[:], selm[:, sj], prodD[:],
                                     start=False, stop=(j == MT - 1),
                                     skip_group_check=True)
                yre_sb = work.tile([12, KC], F16, tag="yre_sb")
                yim_sb = work.tile([12, KC], F16, tag="yim_sb")
                nc.scalar.copy(yre_sb[:], yre[:])
                nc.scalar.copy(yim_sb[:], yim[:])
                zre_ps = psZ.tile([128, KC], F32, tag="zre")
                zim_ps = psZ.tile([128, KC], F32, tag="zim")
                nc.tensor.matmul(zre_ps[:], phit[:], yre_sb[:], start=True, stop=True)
                nc.tensor.matmul(zim_ps[:], phit[:], yim_sb[:], start=True, stop=True)
                nc.scalar.copy(zout_re[:, ks], zre_ps[:])
                nc.scalar.copy(zout_im[:, ks], zim_ps[:])

            nc.gpsimd.dma_start(d_zre[:], zout_re[:])
            nc.gpsimd.dma_start(d_zim[:], zout_im[:])

    nc.finalize()
    return nc


def _get_nc():
    if "nc" not in _CACHE:
        _CACHE["nc"] = _build_nc()
    return _CACHE["nc"]


def _stage_inputs(x, trj, phi, mps, sqrt_dcf):
    """Per-core input maps. Host staging = layout/replication + tiny
    index/scale constants (phase inputs staged in 'turns' with the cos half
    pre-shifted a quarter turn; gy==0 rows use scale=1 with constant input)."""
    f32, f16 = np.float32, np.float16
    gy = np.arange(H, dtype=np.float64) - H // 2
    inv2pi = 1.0 / (2 * np.pi)

    # per-partition scales (col 0: y for 128 rows; col 1: x for 64 rows)
    sc_y = np.where(gy == 0, 1.0, gy * inv2pi)
    pp = np.zeros((128, 2), np.float64)
    pp[:, 0] = np.concatenate([sc_y, sc_y])
    pp[:64, 1] = sc_y

    # cos-half shift: ty + pi/(2*gy) so m_cos = m_sin + 1/4 turn
    with np.errstate(divide="ignore"):
        shift = np.where(gy == 0, 0.0, np.pi / (2 * gy))

    def packed_phase(tv, P):
        """[P, 2, K]: [:,0,:]=tv (sin), [:,1,:]=tv+shift (cos); gy==0 rows
        get constant 0 / 0.25 (scale is 1 there)."""
        g = np.tile(shift, P // H)
        zero = np.tile(gy == 0, P // H)
        out = np.empty((P, 2, K), np.float64)
        out[:, 0, :] = np.where(zero[:, None], 0.0, tv[None, :])
        out[:, 1, :] = np.where(zero[:, None], 0.25, tv[None, :] + g[:, None])
        return out

    # selectors: block j covers ach rows [j*128,(j+1)*128);
    # partition p -> output column ac = 2*j + p//64
    selp = np.zeros((128, 12 * MT), f16)
    for j in range(MT):
        for p in range(128):
            selp[p, j * 12 + 2 * j + p // 64] = 1.0
    sel = np.concatenate([selp, -selp], axis=1)

    phit = np.zeros((12, 128), f16)
    for a in range(A):
        for c in range(C):
            phit[a * 4 + c, c::4] = phi[a].astype(f16)

    xt = np.ascontiguousarray(x.transpose(2, 0, 1))       # [w, a, h]
    xr = np.broadcast_to(xt[:, :, None, :], (W, A, C, H)).reshape(W, ACH)
    mt = np.ascontiguousarray(mps.transpose(2, 0, 1))     # [w, c, h]
    mr = np.broadcast_to(mt[:, None, :, :], (W, A, C, H)).reshape(W, ACH)

    in_maps = []
    for r in range(N_CORES):
        ty = trj[r, 0, :].astype(np.float64)
        tx = trj[r, 1, :].astype(np.float64)
        b64 = np.empty((64, 5632), f32)
        b64[:, 0:2 * K] = packed_phase(tx, 64).reshape(64, 2 * K)
        b64[:, 2 * K:4 * K] = np.broadcast_to(
            sqrt_dcf[r].astype(f32)[None, None, :], (64, 2, K)).reshape(64, 2 * K)
        b64[:, 4 * K:4 * K + ACH] = xr
        b64[:, 4 * K + ACH:] = mr
        b128 = np.empty((128, 2 * K + 2), f32)
        b128[:, 0:2 * K] = packed_phase(ty, 128).reshape(128, 2 * K)
        b128[:, 2 * K:] = pp
        in_maps.append({"b64": b64, "b128": b128, "sel": sel, "phit": phit})
    return in_maps


def kernel(x, trj, phi, mps, sqrt_dcf, subsamp_idx, _trace=False):
    from concourse.bass_utils import run_bass_kernel_spmd

    nc = _get_nc()
    in_maps = _stage_inputs(np.asarray(x), np.asarray(trj), np.asarray(phi),
                            np.asarray(mps), np.asarray(sqrt_dcf))
    res = run_bass_kernel_spmd(nc, in_maps, core_ids=list(range(N_CORES)),
                               trace=_trace)
    out = np.empty((T, C, K), dtype=np.complex64)
    idx = np.asarray(subsamp_idx).astype(np.int64)
    for t in range(T):
        r = int(idx[t])
        zre = res.results[r]["zre"]
        zim = res.results[r]["zim"]
        for c in range(C):
            out[t, c, :] = zre[t * 4 + c] + 1j * zim[t * 4 + c]
    if _trace:
        kernel._last_results = res
    return out



# revision 2
# speedup vs baseline: 1.0245x; 1.0245x over previous
"""Trainium2 Bass kernel v3 for nn_SubspaceLinopFactory (subspace NUDFT).

v3 over v2: single merged phase-matmul per k-block (4 spans), batched input
DMAs (2 tensors + broadcast-DMA'd dcf), fp16 output, outputs via gpsimd DGE.

See kernel_v2.py docstring for the math. Table t5 spans: [sx|cx|sy|cy|nsy].
z spans = [-Im | Re] fp16; host gathers rows 4t+c for r = subsamp_idx[t].
"""
import numpy as np

A, T, C, R, D, K, H, W = 3, 32, 4, 8, 2, 1024, 64, 64
N_CORES = 8
KB = 256
NB = K // KB
MAGIC = float(1.5 * 2 ** 23)
TWO_PI = float(2 * np.pi)
RMB = 11    # merged m-build rows: 3 ty, 3 tx, flagx, 3 ty(y-side), flagy

_CACHE = {}


def _register_frac_op():
    import concourse.dve_ops as dops
    from concourse.dve_spec import Spec, Src0, C0, lower, _has_src1
    from concourse.dve_uop import DveOpSpec

    if "FRAC_ANT" in dops._SUB_OPCODE_FOR_NAME:
        return next(op for op in dops.OPS if op.name == "FRAC_ANT")

    spec = Spec(
        body=Src0 - ((Src0 + C0) - C0),
        reference=lambda in0, in1, s0, s1, imm2: (
            in0.astype(np.float32)
            - ((in0.astype(np.float32) + np.float32(s0)) - np.float32(s0))
        ).astype(np.float32),
    )
    opcode = max(dops._SUB_OPCODE_FOR_NAME.values()) + 1
    shas = {}
    for ver in ("v3", "v4"):
        s = DveOpSpec(name="FRAC_ANT", opcode=opcode, uops=lower(spec, ver=ver),
                      rd1_en=_has_src1(spec))
        shas[ver] = s.sha(ver)
    op = dops.DveOp("FRAC_ANT", spec, subdim=False, uops_sha=shas)
    dops.OPS.append(op)
    dops.CUSTOM_DVE_SPECS["FRAC_ANT"] = spec
    dops._SUB_OPCODE_FOR_NAME["FRAC_ANT"] = opcode
    return op


def _build_nc():
    import concourse.bacc as bacc
    import concourse.tile as tile
    import concourse.mybir as mybir
    from concourse.mybir import VecI64Pair

    AF = mybir.ActivationFunctionType
    OP = mybir.AluOpType
    F32 = mybir.dt.float32
    F16 = mybir.dt.float16
    BF16 = mybir.dt.bfloat16

    frac_op = _register_frac_op()
    nc = bacc.Bacc(None, target_bir_lowering=False)

    # bmb rows 0-6; cols: [cfx 128 | cfy 128 | rfx 2K | rfy 2K] bf16
    d_bmb = nc.dram_tensor("bmb", [7, 256 + 4 * K], BF16, kind="ExternalInput")
    # bfp: [128, 384 (sT) | 384 (wph)] f16
    d_bfp = nc.dram_tensor("bfp", [128, 2 * A * 128], F16, kind="ExternalInput")
    d_dcf = nc.dram_tensor("dcf", [1, 2, K], F16, kind="ExternalInput")
    d_z = nc.dram_tensor("z", [128, 2, K], F16, kind="ExternalOutput")

    with tile.TileContext(nc) as tc:
        with (
            tc.tile_pool(name="cst", bufs=1) as cst,
            tc.tile_pool(name="tbl", bufs=1) as tbl,
            tc.tile_pool(name="fr", bufs=2) as frp,
            tc.tile_pool(name="pq", bufs=3) as pqp,
            tc.tile_pool(name="uvp", bufs=3) as uvp,
            tc.tile_pool(name="psM", bufs=2, space="PSUM") as psM,
            tc.tile_pool(name="psPQ", bufs=2, space="PSUM") as psPQ,
            tc.tile_pool(name="psZ", bufs=2, space="PSUM") as psZ,
        ):
            bmb = cst.tile([7, 256 + 4 * K], BF16)
            bfp = cst.tile([128, 2 * A * 128], F16)
            dcf = cst.tile([128, 2, K], F16)
            nc.sync.dma_start(bmb[:], d_bmb[:])
            nc.sync.dma_start(bfp[:], d_bfp[:])
            nc.sync.dma_start(dcf[:], d_dcf[0:1].partition_broadcast(128))

            cfx = bmb[0:7, 0:128]
            cfy = bmb[0:4, 128:256]
            rfx = bmb[0:7, 256:256 + 2 * K].rearrange("p (s k) -> p s k", k=K)
            rfy = bmb[0:4, 256 + 2 * K:].rearrange("p (s k) -> p s k", k=K)
            sT = bfp[:, 0:A * 128].rearrange("p (a m) -> p a m", m=128)
            wph = bfp[:, A * 128:].rearrange("p (a m) -> p a m", m=128)

            xtd = tbl.tile([128, 2, K], F16)    # [dcf*sx | dcf*cx]
            t5 = tbl.tile([128, 5, K], F16)     # [sx|cx|sy|cy|nsy]
            zout = tbl.tile([128, 2, K], F16)

            def trig(b):
                ks = slice(b * KB, (b + 1) * KB)
                m = psM.tile([128, 4, KB], F32, tag="m")
                nc.tensor.matmul(m[:, 0:2, :], cfx[:], rfx[:, :, ks],
                                 start=True, stop=True)
                nc.tensor.matmul(m[:, 2:4, :], cfy[:], rfy[:, :, ks],
                                 start=True, stop=True)
                fr = frp.tile([128, 4, KB], F32, tag="fr")
                nc.vector._custom_dve(frac_op, out=fr[:], in0=m[:], s0=MAGIC)
                nc.scalar.activation(t5[:, 0:4, ks], fr[:], AF.Sin,
                                     scale=TWO_PI)
                nc.scalar.activation(t5[:, 4, ks], t5[:, 2, ks], AF.Copy,
                                     scale=-1.0)
                nc.vector.tensor_tensor(xtd[:, :, ks], t5[:, 0:2, ks],
                                        dcf[:, :, ks], OP.mult)

            def main(b):
                ks = slice(b * KB, (b + 1) * KB)
                z = psZ.tile([128, 2, KB], F32, tag="z")
                for a in range(A):
                    pq = psPQ.tile([128, 2, KB], F32, tag="pq")
                    nc.tensor.matmul(pq[:], sT[:, a, :], xtd[:, :, ks],
                                     start=True, stop=True)
                    pq16 = pqp.tile([128, 2, KB], F16, tag="pq16")
                    nc.scalar.copy(pq16[:], pq[:])
                    uv = uvp.tile([128, 4, KB], F16, tag="uv")
                    in0 = pq16[:].unsqueeze(2).broadcast_to([128, 2, 2, KB])
                    # in1 spans over t5: [cy(3), nsy(4), sy(2), cy(3)]
                    in1 = t5[:, 3, ks].unsqueeze(1).unsqueeze(1).broadcast_to(
                        [128, 2, 2, KB]).copy()
                    in1.ap = VecI64Pair(
                        [tuple(in1.ap[0]), (-K, 2), (K, 2), (1, KB)])
                    uvv = uv[:].rearrange("p (s d) k -> p s d k", s=2)
                    nc.vector.tensor_tensor(uvv[:], in0[:], in1, OP.mult)
                    nc.tensor.matmul(z[:], wph[:, a, :], uv[:, 0:2, :],
                                     start=(a == 0), stop=False,
                                     skip_group_check=True)
                    nc.tensor.matmul(z[:], wph[:, a, :], uv[:, 2:4, :],
                                     start=False, stop=(a == A - 1),
                                     skip_group_check=True)
                if b % 2 == 0:
                    nc.scalar.copy(zout[:, :, ks], z[:])
                else:
                    nc.vector.tensor_copy(zout[:, :, ks], z[:])
                nc.gpsimd.dma_start(d_z[:, :, ks], zout[:, :, ks])

            trig(0)
            for b in range(NB):
                if b + 1 < NB:
                    trig(b + 1)
                main(b)

    nc.finalize()
    return nc


def _split3(v):
    import ml_dtypes
    bf = ml_dtypes.bfloat16
    h = v.astype(bf)
    m = (v - h.astype(np.float64)).astype(bf)
    l = (v - h.astype(np.float64) - m.astype(np.float64)).astype(bf)
    return h, m, l


def _stage_inputs(x, trj, phi, mps, sqrt_dcf):
    import ml_dtypes
    bf = ml_dtypes.bfloat16
    f16 = np.float16

    s = np.einsum("ahw,chw->achw", x.astype(np.float64), mps.astype(np.float64))
    s = s.reshape(A, C, 2, 32, W)
    sT = np.ascontiguousarray(
        s.transpose(2, 4, 0, 1, 3).reshape(2 * W, A, C * 32)).astype(f16)

    wph = np.zeros((128, A, 128), f16)
    for a in range(A):
        for c in range(C):
            wph[c * 32:(c + 1) * 32, a, np.arange(T) * 4 + c] = \
                phi[a].astype(f16)[None, :]

    bfp = np.concatenate([sT.reshape(128, -1), wph.reshape(128, -1)], axis=1)

    gx = np.arange(W, dtype=np.float64) - 32.0
    cfx = np.zeros((7, 128), np.float64)
    cfx[0:3] = np.repeat([0.0, 32.0], W)[None, :]
    cfx[3:6] = np.tile(gx, 2)[None, :]
    cfx[6] = 1.0
    cfy = np.zeros((7, 128), np.float64)
    cfy[0:3] = np.tile(np.arange(32, dtype=np.float64) - 32.0, 4)[None, :]
    cfy[3] = 1.0

    in_maps = []
    for r in range(N_CORES):
        tys = _split3(trj[r, 0].astype(np.float64) / (2 * np.pi))
        txs = _split3(trj[r, 1].astype(np.float64) / (2 * np.pi))
        rfx = np.zeros((7, 2, K), np.float64)
        rfy = np.zeros((7, 2, K), np.float64)
        for i in range(3):
            rfx[i, 0] = rfx[i, 1] = tys[i].astype(np.float64)
            rfx[3 + i, 0] = rfx[3 + i, 1] = txs[i].astype(np.float64)
            rfy[i, 0] = rfy[i, 1] = tys[i].astype(np.float64)
        rfx[6, 1] = 0.25
        rfy[3, 1] = 0.25
        bmb = np.concatenate(
            [cfx, cfy, rfx.reshape(7, -1), rfy.reshape(7, -1)],
            axis=1).astype(bf)
        in_maps.append({
            "bmb": bmb, "bfp": bfp,
            "dcf": np.ascontiguousarray(
                np.broadcast_to(sqrt_dcf[r].astype(f16)[None, None, :],
                                (1, 2, K))),
        })
    return in_maps


def kernel(x, trj, phi, mps, sqrt_dcf, subsamp_idx, _trace=False):
    from concourse.bass_utils import run_bass_kernel_spmd

    if "nc" not in _CACHE:
        _CACHE["nc"] = _build_nc()
    nc = _CACHE["nc"]
    in_maps = _stage_inputs(np.asarray(x), np.asarray(trj), np.asarray(phi),
                            np.asarray(mps), np.asarray(sqrt_dcf))
    res = run_bass_kernel_spmd(nc, in_maps, core_ids=list(range(N_CORES)),
                               trace=_trace)
    out = np.empty((T, C, K), dtype=np.complex64)
    idx = np.asarray(subsamp_idx).astype(np.int64)
    for t in range(T):
        z = res.results[int(idx[t])]["z"].astype(np.float32)
        rows = z[t * 4: t * 4 + 4]
        out[t, :, :] = rows[:, 1, :] - 1j * rows[:, 0, :]
    if _trace:
        kernel._last_results = res
    return out


# revision 3
# speedup vs baseline: 1.0733x; 1.0475x over previous
"""Trainium2 Bass kernel v3 for nn_SubspaceLinopFactory (subspace NUDFT).

v3 over v2: single merged phase-matmul per k-block (4 spans), batched input
DMAs (2 tensors + broadcast-DMA'd dcf), fp16 output, outputs via gpsimd DGE.

See kernel_v2.py docstring for the math. Table t5 spans: [sx|cx|sy|cy|nsy].
z spans = [-Im | Re] fp16; host gathers rows 4t+c for r = subsamp_idx[t].
"""
import numpy as np

A, T, C, R, D, K, H, W = 3, 32, 4, 8, 2, 1024, 64, 64
N_CORES = 8
KB = 256
NB = K // KB
MAGIC = float(1.5 * 2 ** 23)
TWO_PI = float(2 * np.pi)
RMB = 11    # merged m-build rows: 3 ty, 3 tx, flagx, 3 ty(y-side), flagy

_CACHE = {}


def _register_frac_op():
    import concourse.dve_ops as dops
    from concourse.dve_spec import Spec, Src0, C0, lower, _has_src1
    from concourse.dve_uop import DveOpSpec

    if "FRAC_ANT" in dops._SUB_OPCODE_FOR_NAME:
        return next(op for op in dops.OPS if op.name == "FRAC_ANT")

    spec = Spec(
        body=Src0 - ((Src0 + C0) - C0),
        reference=lambda in0, in1, s0, s1, imm2: (
            in0.astype(np.float32)
            - ((in0.astype(np.float32) + np.float32(s0)) - np.float32(s0))
        ).astype(np.float32),
    )
    opcode = max(dops._SUB_OPCODE_FOR_NAME.values()) + 1
    shas = {}
    for ver in ("v3", "v4"):
        s = DveOpSpec(name="FRAC_ANT", opcode=opcode, uops=lower(spec, ver=ver),
                      rd1_en=_has_src1(spec))
        shas[ver] = s.sha(ver)
    op = dops.DveOp("FRAC_ANT", spec, subdim=False, uops_sha=shas)
    dops.OPS.append(op)
    dops.CUSTOM_DVE_SPECS["FRAC_ANT"] = spec
    dops._SUB_OPCODE_FOR_NAME["FRAC_ANT"] = opcode
    return op


def _build_nc():
    import concourse.bacc as bacc
    import concourse.tile as tile
    import concourse.mybir as mybir
    from concourse.mybir import VecI64Pair

    AF = mybir.ActivationFunctionType
    OP = mybir.AluOpType
    F32 = mybir.dt.float32
    F16 = mybir.dt.float16
    BF16 = mybir.dt.bfloat16

    frac_op = _register_frac_op()
    nc = bacc.Bacc(None, target_bir_lowering=False)

    # bmb rows 0-6; cols: [cfx 128 | cfy 128 | rfx 2K | rfy 2K] bf16
    d_bmb = nc.dram_tensor("bmb", [7, 256 + 4 * K], BF16, kind="ExternalInput")
    # bfp: [128, 384 (sT) | 384 (wph)] f16
    d_bfp = nc.dram_tensor("bfp", [128, 2 * A * 128], F16, kind="ExternalInput")
    d_dcf = nc.dram_tensor("dcf", [1, 2, K], F16, kind="ExternalInput")
    d_z = nc.dram_tensor("z", [128, 2, K], F16, kind="ExternalOutput")

    with tile.TileContext(nc) as tc:
        with (
            tc.tile_pool(name="cst", bufs=1) as cst,
            tc.tile_pool(name="tbl", bufs=1) as tbl,
            tc.tile_pool(name="fr", bufs=2) as frp,
            tc.tile_pool(name="pq", bufs=3) as pqp,
            tc.tile_pool(name="uvp", bufs=3) as uvp,
            tc.tile_pool(name="psM", bufs=2, space="PSUM") as psM,
            tc.tile_pool(name="psPQ", bufs=2, space="PSUM") as psPQ,
            tc.tile_pool(name="psZ", bufs=2, space="PSUM") as psZ,
        ):
            bmb = cst.tile([7, 256 + 4 * K], BF16)
            bfp = cst.tile([128, 2 * A * 128], F16)
            dcf = cst.tile([128, 2, K], F16)
            nc.sync.dma_start(bmb[:], d_bmb[:])
            nc.sync.dma_start(bfp[:], d_bfp[:])
            nc.sync.dma_start(dcf[:], d_dcf[0:1].partition_broadcast(128))

            cfx = bmb[0:7, 0:128]
            cfy = bmb[0:4, 128:256]
            rfx = bmb[0:7, 256:256 + 2 * K].rearrange("p (s k) -> p s k", k=K)
            rfy = bmb[0:4, 256 + 2 * K:].rearrange("p (s k) -> p s k", k=K)
            sT = bfp[:, 0:A * 128].rearrange("p (a m) -> p a m", m=128)
            wph = bfp[:, A * 128:].rearrange("p (a m) -> p a m", m=128)

            t5 = tbl.tile([128, 5, K], F16)     # [sx|cx|sy|cy|nsy]
            zout = tbl.tile([128, 2, K], F16)

            def trig(b):
                ks = slice(b * KB, (b + 1) * KB)
                m = psM.tile([128, 4, KB], F32, tag="m")
                nc.tensor.matmul(m[:, 0:2, :], cfx[:], rfx[:, :, ks],
                                 start=True, stop=True)
                nc.tensor.matmul(m[:, 2:4, :], cfy[:], rfy[:, :, ks],
                                 start=True, stop=True)
                fr = frp.tile([128, 4, KB], F32, tag="fr")
                nc.vector._custom_dve(frac_op, out=fr[:], in0=m[:], s0=MAGIC)
                nc.scalar.activation(t5[:, 0:4, ks], fr[:], AF.Sin,
                                     scale=TWO_PI)

            def neg(b):
                ks = slice(b * KB, (b + 1) * KB)
                nc.scalar.activation(t5[:, 4, ks], t5[:, 2, ks], AF.Copy,
                                     scale=-1.0)

            def main(b):
                ks = slice(b * KB, (b + 1) * KB)
                z = psZ.tile([128, 2, KB], F32, tag="z")
                for a in range(A):
                    pq = psPQ.tile([128, 2, KB], F32, tag="pq")
                    nc.tensor.matmul(pq[:], sT[:, a, :], t5[:, 0:2, ks],
                                     start=True, stop=True)
                    pq16 = pqp.tile([128, 2, KB], F16, tag="pq16")
                    nc.scalar.copy(pq16[:], pq[:])
                    uv = uvp.tile([128, 4, KB], F16, tag="uv")
                    in0 = pq16[:].unsqueeze(2).broadcast_to([128, 2, 2, KB])
                    # in1 spans over t5: [cy(3), nsy(4), sy(2), cy(3)]
                    in1 = t5[:, 3, ks].unsqueeze(1).unsqueeze(1).broadcast_to(
                        [128, 2, 2, KB]).copy()
                    in1.ap = VecI64Pair(
                        [tuple(in1.ap[0]), (-K, 2), (K, 2), (1, KB)])
                    uvv = uv[:].rearrange("p (s d) k -> p s d k", s=2)
                    nc.vector.tensor_tensor(uvv[:], in0[:], in1, OP.mult)
                    nc.tensor.matmul(z[:], wph[:, a, :], uv[:, 0:2, :],
                                     start=(a == 0), stop=False,
                                     skip_group_check=True)
                    nc.tensor.matmul(z[:], wph[:, a, :], uv[:, 2:4, :],
                                     start=False, stop=(a == A - 1),
                                     skip_group_check=True)
                # dcf applied at the end: zout = z * dcf (both spans)
                nc.vector.tensor_tensor(zout[:, :, ks], z[:], dcf[:, :, ks],
                                        OP.mult)
                eng = nc.sync if b == NB - 1 else nc.gpsimd
                eng.dma_start(d_z[:, :, ks], zout[:, :, ks])

            trig(0)
            neg(0)
            for b in range(NB):
                if b + 1 < NB:
                    trig(b + 1)
                main(b)
                if b + 1 < NB:
                    neg(b + 1)

    nc.finalize()
    return nc


def _split3(v):
    import ml_dtypes
    bf = ml_dtypes.bfloat16
    h = v.astype(bf)
    m = (v - h.astype(np.float64)).astype(bf)
    l = (v - h.astype(np.float64) - m.astype(np.float64)).astype(bf)
    return h, m, l


def _stage_inputs(x, trj, phi, mps, sqrt_dcf):
    import ml_dtypes
    bf = ml_dtypes.bfloat16
    f16 = np.float16

    s = np.einsum("ahw,chw->achw", x.astype(np.float64), mps.astype(np.float64))
    s = s.reshape(A, C, 2, 32, W)
    sT = np.ascontiguousarray(
        s.transpose(2, 4, 0, 1, 3).reshape(2 * W, A, C * 32)).astype(f16)

    wph = np.zeros((128, A, 128), f16)
    for a in range(A):
        for c in range(C):
            wph[c * 32:(c + 1) * 32, a, np.arange(T) * 4 + c] = \
                phi[a].astype(f16)[None, :]

    bfp = np.concatenate([sT.reshape(128, -1), wph.reshape(128, -1)], axis=1)

    gx = np.arange(W, dtype=np.float64) - 32.0
    cfx = np.zeros((7, 128), np.float64)
    cfx[0:3] = np.repeat([0.0, 32.0], W)[None, :]
    cfx[3:6] = np.tile(gx, 2)[None, :]
    cfx[6] = 1.0
    cfy = np.zeros((7, 128), np.float64)
    cfy[0:3] = np.tile(np.arange(32, dtype=np.float64) - 32.0, 4)[None, :]
    cfy[3] = 1.0

    in_maps = []
    for r in range(N_CORES):
        tys = _split3(trj[r, 0].astype(np.float64) / (2 * np.pi))
        txs = _split3(trj[r, 1].astype(np.float64) / (2 * np.pi))
        rfx = np.zeros((7, 2, K), np.float64)
        rfy = np.zeros((7, 2, K), np.float64)
        for i in range(3):
            rfx[i, 0] = rfx[i, 1] = tys[i].astype(np.float64)
            rfx[3 + i, 0] = rfx[3 + i, 1] = txs[i].astype(np.float64)
            rfy[i, 0] = rfy[i, 1] = tys[i].astype(np.float64)
        rfx[6, 1] = 0.25
        rfy[3, 1] = 0.25
        bmb = np.concatenate(
            [cfx, cfy, rfx.reshape(7, -1), rfy.reshape(7, -1)],
            axis=1).astype(bf)
        in_maps.append({
            "bmb": bmb, "bfp": bfp,
            "dcf": np.ascontiguousarray(
                np.broadcast_to(sqrt_dcf[r].astype(f16)[None, None, :],
                                (1, 2, K))),
        })
    return in_maps


def kernel(x, trj, phi, mps, sqrt_dcf, subsamp_idx, _trace=False):
    from concourse.bass_utils import run_bass_kernel_spmd

    if "nc" not in _CACHE:
        _CACHE["nc"] = _build_nc()
    nc = _CACHE["nc"]
    in_maps = _stage_inputs(np.asarray(x), np.asarray(trj), np.asarray(phi),
                            np.asarray(mps), np.asarray(sqrt_dcf))
    res = run_bass_kernel_spmd(nc, in_maps, core_ids=list(range(N_CORES)),
                               trace=_trace)
    out = np.empty((T, C, K), dtype=np.complex64)
    idx = np.asarray(subsamp_idx).astype(np.int64)
    for t in range(T):
        z = res.results[int(idx[t])]["z"].astype(np.float32)
        rows = z[t * 4: t * 4 + 4]
        out[t, :, :] = rows[:, 1, :] - 1j * rows[:, 0, :]
    if _trace:
        kernel._last_results = res
    return out


# revision 5
# speedup vs baseline: 1.0825x; 1.0086x over previous
"""Trainium2 Bass kernel v3 for nn_SubspaceLinopFactory (subspace NUDFT).

v3 over v2: single merged phase-matmul per k-block (4 spans), batched input
DMAs (2 tensors + broadcast-DMA'd dcf), fp16 output, outputs via gpsimd DGE.

See kernel_v2.py docstring for the math. Table t5 spans: [sx|cx|sy|cy|nsy].
z spans = [-Im | Re] fp16; host gathers rows 4t+c for r = subsamp_idx[t].
"""
import numpy as np

A, T, C, R, D, K, H, W = 3, 32, 4, 8, 2, 1024, 64, 64
N_CORES = 8
KB = 256
NB = K // KB
MAGIC = float(1.5 * 2 ** 23)
TWO_PI = float(2 * np.pi)
RMB = 8     # merged m-build rows: 2 ty, 2 tx, flagx, 2 ty(y-side), flagy

_CACHE = {}


def _register_frac_op():
    import concourse.dve_ops as dops
    from concourse.dve_spec import Spec, Src0, C0, lower, _has_src1
    from concourse.dve_uop import DveOpSpec

    if "FRAC_ANT" in dops._SUB_OPCODE_FOR_NAME:
        return next(op for op in dops.OPS if op.name == "FRAC_ANT")

    spec = Spec(
        body=Src0 - ((Src0 + C0) - C0),
        reference=lambda in0, in1, s0, s1, imm2: (
            in0.astype(np.float32)
            - ((in0.astype(np.float32) + np.float32(s0)) - np.float32(s0))
        ).astype(np.float32),
    )
    opcode = max(dops._SUB_OPCODE_FOR_NAME.values()) + 1
    shas = {}
    for ver in ("v3", "v4"):
        s = DveOpSpec(name="FRAC_ANT", opcode=opcode, uops=lower(spec, ver=ver),
                      rd1_en=_has_src1(spec))
        shas[ver] = s.sha(ver)
    op = dops.DveOp("FRAC_ANT", spec, subdim=False, uops_sha=shas)
    dops.OPS.append(op)
    dops.CUSTOM_DVE_SPECS["FRAC_ANT"] = spec
    dops._SUB_OPCODE_FOR_NAME["FRAC_ANT"] = opcode
    return op


def _build_nc():
    import concourse.bacc as bacc
    import concourse.tile as tile
    import concourse.mybir as mybir
    from concourse.mybir import VecI64Pair

    AF = mybir.ActivationFunctionType
    OP = mybir.AluOpType
    F32 = mybir.dt.float32
    F16 = mybir.dt.float16
    BF16 = mybir.dt.bfloat16

    frac_op = _register_frac_op()
    nc = bacc.Bacc(None, target_bir_lowering=False)

    # bmb rows 0-6; cols: [cfx 128 | cfy 128 | rfx 2K | rfy 2K] bf16
    d_bmb = nc.dram_tensor("bmb", [5, 256 + 4 * K], BF16, kind="ExternalInput")
    # bfp: [128, 384 (sT) | 384 (wph)] f16
    d_bfp = nc.dram_tensor("bfp", [128, 2 * A * 128], F16, kind="ExternalInput")
    d_dcf = nc.dram_tensor("dcf", [1, 2, K], F16, kind="ExternalInput")
    d_z = nc.dram_tensor("z", [128, 2, K], F16, kind="ExternalOutput")

    with tile.TileContext(nc) as tc:
        with (
            tc.tile_pool(name="cst", bufs=1) as cst,
            tc.tile_pool(name="tbl", bufs=1) as tbl,
            tc.tile_pool(name="fr", bufs=2) as frp,
            tc.tile_pool(name="pq", bufs=3) as pqp,
            tc.tile_pool(name="uvp", bufs=3) as uvp,
            tc.tile_pool(name="psM", bufs=2, space="PSUM") as psM,
            tc.tile_pool(name="psPQ", bufs=2, space="PSUM") as psPQ,
            tc.tile_pool(name="psZ", bufs=2, space="PSUM") as psZ,
        ):
            bmb = cst.tile([5, 256 + 4 * K], BF16)
            bfp = cst.tile([128, 2 * A * 128], F16)
            dcf = cst.tile([128, 2, K], F16)
            nc.sync.dma_start(bmb[:], d_bmb[:])
            nc.sync.dma_start(bfp[:], d_bfp[:])
            nc.sync.dma_start(dcf[:], d_dcf[0:1].partition_broadcast(128))

            cfx = bmb[0:5, 0:128]
            cfy = bmb[0:3, 128:256]
            rfx = bmb[0:5, 256:256 + 2 * K].rearrange("p (s k) -> p s k", k=K)
            rfy = bmb[0:3, 256 + 2 * K:].rearrange("p (s k) -> p s k", k=K)
            sT = bfp[:, 0:A * 128].rearrange("p (a m) -> p a m", m=128)
            wph = bfp[:, A * 128:].rearrange("p (a m) -> p a m", m=128)

            t5 = tbl.tile([128, 5, K], F16)     # [sx|cx|sy|cy|nsy]
            zout = tbl.tile([128, 2, K], F16)

            def trig(b):
                ks = slice(b * KB, (b + 1) * KB)
                m = psM.tile([128, 4, KB], F32, tag="m")
                nc.tensor.matmul(m[:, 0:2, :], cfx[:], rfx[:, :, ks],
                                 start=True, stop=True)
                nc.tensor.matmul(m[:, 2:4, :], cfy[:], rfy[:, :, ks],
                                 start=True, stop=True)
                fr = frp.tile([128, 4, KB], F32, tag="fr")
                nc.vector._custom_dve(frac_op, out=fr[:], in0=m[:], s0=MAGIC)
                nc.scalar.activation(t5[:, 0:4, ks], fr[:], AF.Sin,
                                     scale=TWO_PI)

            def neg(b):
                ks = slice(b * KB, (b + 1) * KB)
                nc.scalar.activation(t5[:, 4, ks], t5[:, 2, ks], AF.Copy,
                                     scale=-1.0)

            def main(b):
                ks = slice(b * KB, (b + 1) * KB)
                z = psZ.tile([128, 2, KB], F32, tag="z")
                for a in range(A):
                    pq = psPQ.tile([128, 2, KB], F32, tag="pq")
                    nc.tensor.matmul(pq[:], sT[:, a, :], t5[:, 0:2, ks],
                                     start=True, stop=True)
                    pq16 = pqp.tile([128, 2, KB], F16, tag="pq16")
                    nc.scalar.copy(pq16[:], pq[:])
                    if a == 0 and b + 1 < NB:
                        trig(b + 1)
                    uv = uvp.tile([128, 4, KB], F16, tag="uv")
                    in0 = pq16[:].unsqueeze(2).broadcast_to([128, 2, 2, KB])
                    # in1 spans over t5: [cy(3), nsy(4), sy(2), cy(3)]
                    in1 = t5[:, 3, ks].unsqueeze(1).unsqueeze(1).broadcast_to(
                        [128, 2, 2, KB]).copy()
                    in1.ap = VecI64Pair(
                        [tuple(in1.ap[0]), (-K, 2), (K, 2), (1, KB)])
                    uvv = uv[:].rearrange("p (s d) k -> p s d k", s=2)
                    nc.vector.tensor_tensor(uvv[:], in0[:], in1, OP.mult)
                    nc.tensor.matmul(z[:], wph[:, a, :], uv[:, 0:2, :],
                                     start=(a == 0), stop=False,
                                     skip_group_check=True)
                    nc.tensor.matmul(z[:], wph[:, a, :], uv[:, 2:4, :],
                                     start=False, stop=(a == A - 1),
                                     skip_group_check=True)
                # dcf applied at the end: zout = z * dcf (both spans)
                nc.vector.tensor_tensor(zout[:, :, ks], z[:], dcf[:, :, ks],
                                        OP.mult)
                eng = nc.sync if b == NB - 1 else nc.gpsimd
                eng.dma_start(d_z[:, :, ks], zout[:, :, ks])

            trig(0)
            neg(0)
            for b in range(NB):
                main(b)
                if b + 1 < NB:
                    neg(b + 1)

    nc.finalize()
    return nc


def _split2(v):
    import ml_dtypes
    bf = ml_dtypes.bfloat16
    h = v.astype(bf)
    m = (v - h.astype(np.float64)).astype(bf)
    return h, m


def _stage_inputs(x, trj, phi, mps, sqrt_dcf):
    import ml_dtypes
    bf = ml_dtypes.bfloat16
    f16 = np.float16

    s = np.einsum("ahw,chw->achw", x.astype(np.float64), mps.astype(np.float64))
    s = s.reshape(A, C, 2, 32, W)
    sT = np.ascontiguousarray(
        s.transpose(2, 4, 0, 1, 3).reshape(2 * W, A, C * 32)).astype(f16)

    wph = np.zeros((128, A, 128), f16)
    for a in range(A):
        for c in range(C):
            wph[c * 32:(c + 1) * 32, a, np.arange(T) * 4 + c] = \
                phi[a].astype(f16)[None, :]

    bfp = np.concatenate([sT.reshape(128, -1), wph.reshape(128, -1)], axis=1)

    gx = np.arange(W, dtype=np.float64) - 32.0
    cfx = np.zeros((5, 128), np.float64)
    cfx[0:2] = np.repeat([0.0, 32.0], W)[None, :]
    cfx[2:4] = np.tile(gx, 2)[None, :]
    cfx[4] = 1.0
    cfy = np.zeros((5, 128), np.float64)
    cfy[0:2] = np.tile(np.arange(32, dtype=np.float64) - 32.0, 4)[None, :]
    cfy[2] = 1.0

    in_maps = []
    for r in range(N_CORES):
        tys = _split2(trj[r, 0].astype(np.float64) / (2 * np.pi))
        txs = _split2(trj[r, 1].astype(np.float64) / (2 * np.pi))
        rfx = np.zeros((5, 2, K), np.float64)
        rfy = np.zeros((5, 2, K), np.float64)
        for i in range(2):
            rfx[i, 0] = rfx[i, 1] = tys[i].astype(np.float64)
            rfx[2 + i, 0] = rfx[2 + i, 1] = txs[i].astype(np.float64)
            rfy[i, 0] = rfy[i, 1] = tys[i].astype(np.float64)
        rfx[4, 1] = 0.25
        rfy[2, 1] = 0.25
        bmb = np.concatenate(
            [cfx, cfy, rfx.reshape(5, -1), rfy.reshape(5, -1)],
            axis=1).astype(bf)
        in_maps.append({
            "bmb": bmb, "bfp": bfp,
            "dcf": np.ascontiguousarray(
                np.broadcast_to(sqrt_dcf[r].astype(f16)[None, None, :],
                                (1, 2, K))),
        })
    return in_maps


def kernel(x, trj, phi, mps, sqrt_dcf, subsamp_idx, _trace=False):
    from concourse.bass_utils import run_bass_kernel_spmd

    if "nc" not in _CACHE:
        _CACHE["nc"] = _build_nc()
    nc = _CACHE["nc"]
    in_maps = _stage_inputs(np.asarray(x), np.asarray(trj), np.asarray(phi),
                            np.asarray(mps), np.asarray(sqrt_dcf))
    res = run_bass_kernel_spmd(nc, in_maps, core_ids=list(range(N_CORES)),
                               trace=_trace)
    out = np.empty((T, C, K), dtype=np.complex64)
    idx = np.asarray(subsamp_idx).astype(np.int64)
    for t in range(T):
        z = res.results[int(idx[t])]["z"].astype(np.float32)
        rows = z[t * 4: t * 4 + 4]
        out[t, :, :] = rows[:, 1, :] - 1j * rows[:, 0, :]
    if _trace:
        kernel._last_results = res
    return out
